# revision 1
# baseline (speedup 1.0000x reference)
"""CRAM block Trainium2 kernel (Bass/Tile), 8-core SPMD.

Shard: core i -> (batch b=i//2, seq-half i%2): T=2048 tokens + 128-token halo.

Phases (per core):
  A1: per 128-token chunk: x -> PE-transpose -> x^T(f32r) -> sig =
      sigmoid(x@W_ret + b_ret) (activation-stationary, token-major out)
      -> EMA via L/U triangular-matmuls -> LN1(x+r) = h -> spill h to DRAM.
  A2: reload h chunks, PE-transpose into resident h^T [H, T] (f32r);
      overlaps W1 load.
  B:  g^T = gelu(W1^T h^T + b1) feature-major, W1-stationary k-amortized
      over 4 PSUM banks; g spilled to DRAM blocked [tt, fc, tsub, f, t].
  C:  per 128-token chunk: ffn = g@W2 (g-tile-stationary, W2 moving,
      token-major out) + b2 + h residual -> LN2 -> out.

EMA-as-matmul: decay 0.5 => contributions >256 steps back are below fp32
resolution; r_chunk = L @ sig_chunk + U @ sig_prev_chunk with
L^T[j,t]=0.5^(t-j+1) (j<=t), U^T[j,t]=0.5^(t+129-j). Cores at seq start
get U=0 (no halo); halo x rows are garbage there by construction.
"""
import sys
sys.path.insert(0, '/opt/trn_rl_repo')

from contextlib import ExitStack

import numpy as np
import concourse.bass as bass
import concourse.tile as tile
from concourse import mybir, bacc
import time
import jax
from jax.sharding import Mesh, PartitionSpec
from jax.experimental.shard_map import shard_map
from concourse.bass2jax import _bass_exec_p, partition_id_tensor, install_neuronx_cc_hook


F32 = mybir.dt.float32
F32R = mybir.dt.float32r
AF = mybir.ActivationFunctionType

B, S, H, FF = 4, 4096, 1024, 4096
EPS = 1e-5
N_CORES = 8
T = 2048            # tokens per core
TC = T // 128       # 16 output chunks
TCI = TC + 1        # incl. halo chunk
KH = H // 128       # 8 h chunks
KF = FF // 128      # 32 f chunks
NT = T // 512       # 4 t-tiles of 512
GELU = AF.Gelu_apprx_tanh   # jax.nn.gelu default is approximate=True (tanh)


def build_nc(repeat=1, debug_taps=False):
    nc = bacc.Bacc("TRN2", target_bir_lowering=False, debug=False,
                   num_devices=N_CORES)

    x_in = nc.dram_tensor("x", [TCI * 128, H], F32, kind="ExternalInput")
    wret_in = nc.dram_tensor("wret", [H, H], F32R, kind="ExternalInput")
    w1_in = nc.dram_tensor("w1", [KF, 128, KH * 128], F32R, kind="ExternalInput")
    w2_in = nc.dram_tensor("w2", [FF, H], F32R, kind="ExternalInput")
    bret_in = nc.dram_tensor("bret", [128, H], F32, kind="ExternalInput")
    b2_in = nc.dram_tensor("b2", [128, H], F32, kind="ExternalInput")
    lns1_in = nc.dram_tensor("lns1", [128, H], F32, kind="ExternalInput")
    lnb1_in = nc.dram_tensor("lnb1", [128, H], F32, kind="ExternalInput")
    lns2_in = nc.dram_tensor("lns2", [128, H], F32, kind="ExternalInput")
    lnb2_in = nc.dram_tensor("lnb2", [128, H], F32, kind="ExternalInput")
    b1_in = nc.dram_tensor("b1", [128, KF], F32, kind="ExternalInput")
    ema_l_in = nc.dram_tensor("ema_l", [128, 128], F32R, kind="ExternalInput")
    ema_u_in = nc.dram_tensor("ema_u", [128, 128], F32R, kind="ExternalInput")
    ema_u0_in = nc.dram_tensor("ema_u0", [128, 128], F32R, kind="ExternalInput")
    ident_in = nc.dram_tensor("ident", [128, 128], F32, kind="ExternalInput")

    out_t = nc.dram_tensor("out", [T, H], F32, kind="ExternalOutput")

    ins = dict(x=x_in, wret=wret_in, w1=w1_in, w2=w2_in, bret=bret_in,
               b2=b2_in, lns1=lns1_in, lnb1=lnb1_in, lns2=lns2_in,
               lnb2=lnb2_in, b1=b1_in, ema_l=ema_l_in, ema_u=ema_u_in,
               ema_u0=ema_u0_in, ident=ident_in)
    if debug_taps:
        ins["h_dbg"] = nc.dram_tensor("h_dbg", [TC, 128, H], F32, kind="ExternalOutput")
        ins["g_dbg"] = nc.dram_tensor("g_dbg", [NT, KF, 4, 128, 128], F32, kind="ExternalOutput")
        ins["sig_dbg"] = nc.dram_tensor("sig_dbg", [TCI, 128, H], F32, kind="ExternalOutput")

    with tile.TileContext(nc) as tc:
        with ExitStack() as octx:
            singles = octx.enter_context(tc.tile_pool(name="singles", bufs=1))
            cst = load_constants(tc, singles, ins)
            for _ in range(repeat):
                one_pass(tc, cst, ins, out_t, debug_taps=debug_taps)
    nc.compile()
    return nc


def load_constants(tc, singles, ins):
    nc = tc.nc
    cst = {}

    def load(name, shape, dt, src):
        t = singles.tile(shape, dt, name=name, tag=name)
        nc.sync.dma_start(out=t[:], in_=src)
        cst[name] = t
        return t

    load("ident", [128, 128], F32, ins["ident"][:])
    load("ema_l", [128, 128], F32R, ins["ema_l"][:])
    load("ema_u", [128, 128], F32R, ins["ema_u"][:])
    load("ema_u0", [128, 128], F32R, ins["ema_u0"][:])
    for nm in ["bret", "b2", "lns1", "lnb1", "lns2", "lnb2"]:
        load(nm, [128, H], F32, ins[nm][:])
    load("b1", [128, KF], F32, ins["b1"][:])
    eps_t = singles.tile([128, 1], F32)
    nc.vector.memset(eps_t[:], EPS)
    cst["eps"] = eps_t
    return cst


def one_pass(tc, cst, ins, out_t, debug_taps=False):
    nc = tc.nc
    with ExitStack() as octx:
        # DRAM scratch, dependency-tracked via DRAM pool
        dram = octx.enter_context(tc.tile_pool(name="dram", bufs=1, space="DRAM"))
        h_scr = dram.tile([TC, 128, H], F32)
        g_scr = dram.tile([NT, KF, 4, 128, 128], F32R)

        if True:
            # ---------------- Phase A1 ----------------
            ab = ExitStack()
            hT_pool = ab.enter_context(tc.tile_pool(name="hT", bufs=1))
            hT = hT_pool.tile([128, KH, T], F32R)
            with ExitStack() as ctx:
                wret_pool = ctx.enter_context(tc.tile_pool(name="wretp", bufs=1))
                wret_sb = wret_pool.tile([128, KH, H], F32R)
                for k in range(KH):
                    nc.sync.dma_start(out=wret_sb[:, k, :],
                                      in_=ins["wret"][k * 128:(k + 1) * 128, :])

                pa = ctx.enter_context(tc.tile_pool(name="pa", bufs=2))
                pa_sig = ctx.enter_context(tc.tile_pool(name="pa_sig", bufs=4))
                ps_t = ctx.enter_context(tc.tile_pool(name="ps_t", bufs=2, space="PSUM"))
                ps_sig = ctx.enter_context(tc.tile_pool(name="ps_sig", bufs=1, space="PSUM"))
                ps_r = ctx.enter_context(tc.tile_pool(name="ps_r", bufs=2, space="PSUM"))

                sig_prev = None
                for c in range(TCI):
                    xc = pa.tile([128, H], F32, tag="xc")
                    nc.sync.dma_start(out=xc[:], in_=ins["x"][c * 128:(c + 1) * 128, :])
                    xT = pa.tile([128, KH, 128], F32R, tag="xT")
                    for k in range(KH):
                        pt = ps_t.tile([128, 128], F32, tag="pt")
                        nc.tensor.transpose(pt[:], xc[:, k * 128:(k + 1) * 128], cst["ident"][:])
                        nc.vector.tensor_copy(out=xT[:, k, :], in_=pt[:])
                    psig = ps_sig.tile([128, H], F32, tag="psig")
                    for k in range(KH):
                        for n in range(2):
                            nc.tensor.matmul(
                                psig[:, n * 512:(n + 1) * 512],
                                xT[:, k, :],
                                wret_sb[:, k, n * 512:(n + 1) * 512],
                                start=(k == 0), stop=(k == KH - 1),
                                skip_group_check=True,
                            )
                    nc.vector.tensor_add(out=psig[:], in0=psig[:], in1=cst["bret"][:])
                    sig = pa_sig.tile([128, H], F32R, tag="sig")
                    nc.scalar.activation(out=sig[:], in_=psig[:], func=AF.Sigmoid)
                    if debug_taps:
                        nc.sync.dma_start(
                            out=ins["sig_dbg"][c], in_=sig[:].bitcast(F32))

                    if c > 0:
                        pr = ps_r.tile([128, H], F32, tag="pr")
                        for n in range(2):
                            sl = slice(n * 512, (n + 1) * 512)
                            nc.tensor.matmul(pr[:, sl], cst["ema_l"][:], sig[:, sl],
                                             start=True, stop=False, skip_group_check=True)
                        uu = cst["ema_u0"] if c == 1 else cst["ema_u"]
                        for n in range(2):
                            sl = slice(n * 512, (n + 1) * 512)
                            nc.tensor.matmul(pr[:, sl], uu[:], sig_prev[:, sl],
                                             start=False, stop=True, skip_group_check=True)
                        v = pa.tile([128, H], F32, tag="v")
                        nc.vector.tensor_add(out=v[:], in0=pr[:], in1=xc[:])
                        hc = pa.tile([128, H], F32, tag="hc")
                        layernorm(nc, pa, v, hc, cst["eps"], cst["lns1"], cst["lnb1"])
                        nc.sync.dma_start(out=h_scr[c - 1], in_=hc[:])
                        for k in range(KH):
                            pt = ps_t.tile([128, 128], F32, tag="pt")
                            nc.tensor.transpose(pt[:], hc[:, k * 128:(k + 1) * 128],
                                                cst["ident"][:])
                            nc.vector.tensor_copy(
                                out=hT[:, k, (c - 1) * 128:c * 128], in_=pt[:])
                    sig_prev = sig

            # ---------------- Phase B (W1 streamed per f-block) ----------------
            with ExitStack() as ctx:
                pb_w1 = ctx.enter_context(tc.tile_pool(name="pb_w1", bufs=4))
                pb_g = ctx.enter_context(tc.tile_pool(name="pb_g", bufs=6))
                ps_g = ctx.enter_context(tc.tile_pool(name="ps_g", bufs=2, space="PSUM"))
                for f in range(KF):
                    w1t = pb_w1.tile([128, KH, 128], F32R, tag="w1t")
                    nc.sync.dma_start(out=w1t[:], in_=ins["w1"][f])
                    pg = ps_g.tile([128, NT, 512], F32, tag="pg")
                    for k in range(KH):
                        for tt in range(NT):
                            nc.tensor.matmul(
                                pg[:, tt, :],
                                w1t[:, k, :],
                                hT[:, k, tt * 512:(tt + 1) * 512],
                                start=(k == 0), stop=(k == KH - 1),
                                skip_group_check=True,
                            )
                    for tt in range(NT):
                        g = pb_g.tile([128, 512], F32R, tag="g")
                        nc.scalar.activation(out=g[:], in_=pg[:, tt, :], func=globals()['GELU'],
                                             bias=cst["b1"][:, f:f + 1], scale=1.0)
                        for s4 in range(4):
                            nc.sync.dma_start(out=g_scr[tt, f, s4],
                                              in_=g[:, s4 * 128:(s4 + 1) * 128])
                        if debug_taps:
                            for s4 in range(4):
                                nc.sync.dma_start(out=ins["g_dbg"][tt, f, s4],
                                                  in_=g[:, s4 * 128:(s4 + 1) * 128].bitcast(F32))

        ab.close()  # free hT before W2 loads
        # ---------------- Phase C ----------------
        with ExitStack() as ctx:
            pc_w2 = ctx.enter_context(tc.tile_pool(name="pc_w2", bufs=1))
            w2_sb = pc_w2.tile([128, KF, H], F32R)
            for k in range(KF):
                nc.sync.dma_start(out=w2_sb[:, k, :],
                                  in_=ins["w2"][k * 128:(k + 1) * 128, :])

            pc = ctx.enter_context(tc.tile_pool(name="pc", bufs=2))
            pc_g = ctx.enter_context(tc.tile_pool(name="pc_g", bufs=12))
            ps_c = ctx.enter_context(tc.tile_pool(name="ps_c", bufs=4, space="PSUM"))

            for t in range(TC):
                tt, tsub = divmod(t, 4)
                pcs = ps_c.tile([128, H], F32, tag="pcs")
                for f in range(KF):
                    gt = pc_g.tile([128, 128], F32R, tag="gt")
                    nc.sync.dma_start(out=gt[:], in_=g_scr[tt, f, tsub])
                    for n in range(2):
                        nc.tensor.matmul(
                            pcs[:, n * 512:(n + 1) * 512],
                            gt[:],
                            w2_sb[:, f, n * 512:(n + 1) * 512],
                            start=(f == 0), stop=(f == KF - 1),
                            skip_group_check=True,
                        )
                hc = pc.tile([128, H], F32, tag="hc3")
                nc.sync.dma_start(out=hc[:], in_=h_scr[t])
                if debug_taps:
                    nc.sync.dma_start(out=ins["h_dbg"][t], in_=hc[:])
                v = pc.tile([128, H], F32, tag="v2")
                nc.vector.tensor_add(out=v[:], in0=pcs[:], in1=hc[:])
                nc.vector.tensor_add(out=v[:], in0=v[:], in1=cst["b2"][:])
                o = pc.tile([128, H], F32, tag="o")
                layernorm(nc, pc, v, o, cst["eps"], cst["lns2"], cst["lnb2"])
                nc.sync.dma_start(out=out_t[t * 128:(t + 1) * 128, :], in_=o[:])


def layernorm(nc, pool, v, out, eps_t, scale_bc, bias_bc):
    """out = (v - mean)/sqrt(var+eps) * scale + bias over free dim (H)."""
    stats = pool.tile([128, 2, 6], F32, tag="ln_stats")
    v2 = v[:].rearrange("p (s q) -> p s q", s=2)
    for s in range(2):
        nc.vector.bn_stats(out=stats[:, s, :], in_=v2[:, s, :])
    mv = pool.tile([128, 2], F32, tag="ln_mv")
    nc.vector.bn_aggr(out=mv[:], in_=stats[:])
    std = pool.tile([128, 1], F32, tag="ln_std")
    nc.scalar.activation(out=std[:], in_=mv[:, 1:2], func=AF.Sqrt,
                         bias=eps_t[:], scale=1.0)
    rstd = pool.tile([128, 1], F32, tag="ln_rstd")
    nc.vector.reciprocal(out=rstd[:], in_=std[:])
    nc.vector.tensor_scalar(out=out[:], in0=v[:], scalar1=mv[:, 0:1],
                            scalar2=rstd[:],
                            op0=mybir.AluOpType.subtract, op1=mybir.AluOpType.mult)
    nc.vector.tensor_mul(out=out[:], in0=out[:], in1=scale_bc[:])
    nc.vector.tensor_add(out=out[:], in0=out[:], in1=bias_bc[:])


# ---------------------------------------------------------------------------
# Host side
# ---------------------------------------------------------------------------

def make_ema_mats():
    t = np.arange(128)
    j = np.arange(128)[:, None]
    Lt = np.where(j <= t[None, :], 0.5 ** (t[None, :] - j + 1.0), 0.0)
    Ut = 0.5 ** (t[None, :] + 129.0 - j)
    return Lt.astype(np.float32), Ut.astype(np.float32)


def make_in_maps(x, W_ret, b_ret, ln1_scale, ln1_bias, W1, b1, W2, b2,
                 ln2_scale, ln2_bias):
    Lt, Ut = make_ema_mats()
    bc = lambda vec: np.ascontiguousarray(
        np.broadcast_to(np.asarray(vec, np.float32)[None, :], (128, len(vec))))
    common = {
        "wret": np.ascontiguousarray(W_ret, np.float32),
        "w1": np.ascontiguousarray(
            np.asarray(W1, np.float32).reshape(KH, 128, KF, 128)
            .transpose(2, 1, 0, 3).reshape(KF, 128, KH * 128)),
        "w2": np.ascontiguousarray(W2, np.float32),
        "bret": bc(b_ret), "b2": bc(b2),
        "lns1": bc(ln1_scale), "lnb1": bc(ln1_bias),
        "lns2": bc(ln2_scale), "lnb2": bc(ln2_bias),
        "b1": np.ascontiguousarray(np.asarray(b1, np.float32).reshape(KF, 128).T),
        "ema_l": Lt,
        "ident": np.eye(128, dtype=np.float32),
    }
    in_maps = []
    for core in range(N_CORES):
        b, half = divmod(core, 2)
        xs = np.empty((TCI * 128, H), np.float32)
        if half == 0:
            xs[:128] = 0.0
            xs[128:] = x[b, 0:T]
            U = np.zeros_like(Ut)
        else:
            xs[:] = x[b, T - 128:S]
            U = Ut
        m = dict(common)
        m["x"] = xs
        m["ema_u"] = Ut
        m["ema_u0"] = U
        in_maps.append(m)
    return in_maps


def gather_out(results):
    out = np.empty((B, S, H), np.float32)
    for core in range(N_CORES):
        b, half = divmod(core, 2)
        out[b, half * T:(half + 1) * T] = results[core]["out"]
    return out


class SpmdRunner:
    def __init__(self, nc, n_cores):
        install_neuronx_cc_hook()
        self.nc = nc
        self.n_cores = n_cores
        assert nc.dbg_addr is None or not nc.dbg_callbacks

        in_names, out_names, out_avals, zero_outs = [], [], [], []
        partition_name = nc.partition_id_tensor.name if nc.partition_id_tensor else None
        for alloc in nc.m.functions[0].allocations:
            if not isinstance(alloc, mybir.MemoryLocationSet):
                continue
            name = alloc.memorylocations[0].name
            if alloc.kind == "ExternalInput":
                if name != partition_name:
                    in_names.append(name)
            elif alloc.kind == "ExternalOutput":
                shape = tuple(alloc.tensor_shape)
                dtype = mybir.dt.np(alloc.dtype)
                out_names.append(name)
                out_avals.append(jax.core.ShapedArray(shape, dtype))
                zero_outs.append(np.zeros(shape, dtype))
        if nc.dbg_addr is not None:
            # supply zeroed dbg_addr (see bass2jax run_bass_via_pjrt)
            self.dbg_name = nc.dbg_addr.name
        else:
            self.dbg_name = None
        self.in_names = list(in_names)
        self.out_names = out_names
        self.out_avals = out_avals
        self.zero_outs = zero_outs
        self.partition_name = partition_name
        n_params = len(self.in_names)
        n_outs = len(out_names)

        all_in_names = list(self.in_names) + list(out_names)
        if partition_name is not None:
            all_in_names.append(partition_name)

        def _body(*args):
            operands = list(args)
            if partition_name is not None:
                operands.append(partition_id_tensor())
            outs = _bass_exec_p.bind(
                *operands,
                out_avals=tuple(out_avals),
                in_names=tuple(all_in_names),
                out_names=tuple(out_names),
                lowering_input_output_aliases=(),
                sim_require_finite=True,
                sim_require_nnan=True,
                nc=nc,
            )
            return tuple(outs)

        devices = jax.devices()[:n_cores]
        assert len(devices) == n_cores
        self.mesh = Mesh(np.asarray(devices), ("core",))
        in_specs = (PartitionSpec("core"),) * (n_params + n_outs)
        out_specs = (PartitionSpec("core"),) * n_outs
        # no donation: lets us reuse the same zero buffers across timing calls
        self.fn = jax.jit(
            shard_map(_body, mesh=self.mesh, in_specs=in_specs,
                      out_specs=out_specs, check_rep=False),
            keep_unused=True,
        )
        self._dev_zeros = None

    def _concat(self, in_maps):
        per_core = [[np.asarray(m[name]) for name in self.in_names] for m in in_maps]
        return [np.concatenate([per_core[c][i] for c in range(self.n_cores)], axis=0)
                for i in range(len(self.in_names))]

    def put(self, in_maps):
        """device_put concatenated inputs; returns device args usable in run()."""
        concat_in = self._concat(in_maps)
        dev_in = [jax.device_put(x) for x in concat_in]
        if self._dev_zeros is None:
            self._dev_zeros = [
                jax.device_put(np.zeros((self.n_cores * z.shape[0], *z.shape[1:]), z.dtype))
                for z in self.zero_outs
            ]
        return dev_in

    def run(self, dev_in):
        out = self.fn(*dev_in, *self._dev_zeros)
        jax.block_until_ready(out)
        return out

    def results(self, out_arrs):
        res = []
        for c in range(self.n_cores):
            res.append({
                name: np.asarray(out_arrs[i]).reshape(self.n_cores, *self.out_avals[i].shape)[c]
                for i, name in enumerate(self.out_names)
            })
        return res

    def time_exec(self, dev_in, n=5):
        ts = []
        for _ in range(n):
            t0 = time.perf_counter()
            self.run(dev_in)
            ts.append(time.perf_counter() - t0)
        return min(ts), ts


# ---------------------------------------------------------------------------
# Public entry point: full inputs in, full output out.
# ---------------------------------------------------------------------------

_CACHE = {}


def kernel(x, W_ret, b_ret, ln1_scale, ln1_bias, W1, b1, W2, b2,
           ln2_scale, ln2_bias):
    """CRAM block on 8 Trainium2 NeuronCores. Full [4,4096,1024] in/out."""
    if "runner" not in _CACHE:
        nc = build_nc(repeat=1)
        _CACHE["runner"] = SpmdRunner(nc, N_CORES)
    runner = _CACHE["runner"]
    in_maps = make_in_maps(x, W_ret, b_ret, ln1_scale, ln1_bias, W1, b1,
                           W2, b2, ln2_scale, ln2_bias)
    dev_in = runner.put(in_maps)
    results = runner.results(runner.run(dev_in))
    return gather_out(results).astype(np.float32)

